# revision 15
# baseline (speedup 1.0000x reference)
"""Multi-head attention forward on 8 Trainium2 NeuronCores.

Problem: x[4,2048,1024], 16 heads (d=64), fp32. out = softmax(QK^T/sqrt(d) + mask) V @ Wo.

Sharding: core = (batch b in 0..3) x (head-group hg in 0..1). Each core handles one
batch element and 8 heads (a 512-wide slice of the model dim). Each core emits a
partial output [2048,1024] (its heads' contribution through Wo); the host sums the
two head-group partials per batch element.

All matmuls run in float32r (full-rate fp32 on the PE for moving dim >= 256). The
BIR verifier requires fp32r matmul operands to be *typed* fp32r at their producing
instruction, so every producer (DMA loads, DVE copies, ACT exp) writes through a
bitcast AP.

Per-core pipeline:
  A) QT,KT = projections in transposed layout [512,2048] (heads pair-packed along
     partitions); V in natural layout, head-interleaved with a ones column per head
     ([128, 8*65]) so the attention matmul also produces the softmax row sums.
  B) per head-pair (2m, 2m+1), per 512-query tile, per 128-key chunk: two K=64
     logits matmuls land in the two halves of a [128,1024] PSUM tile (the two
     heads sit in PE row-groups 0-1/2-3 and run concurrently); one ACT exp over
     [128,1024] with the key mask as per-partition bias and 1/sqrt(d) folded into
     the activation scale; two PT @ V_aug accumulations -> attn_aug[65,512] per
     head (row 64 = exp row sum). Then per head: DVE reciprocal of the row-sum
     row, K=1 outer-product matmul broadcasts it over 64 partitions, DVE multiply
     writes normalized attn^T into SBUF.
  C) out[q,:] = attnT-chunk^T @ Wo-slice (K=512 accumulation), DVE copy, DMA out.
"""
import sys

sys.path.insert(0, "/opt/trn_rl_repo")
sys.path.insert(0, "/root/problem")

import numpy as np

import concourse.bass as bass
import concourse.tile as tile
from concourse import library_config, mybir
from concourse.bass_utils import run_bass_kernel_spmd
from concourse.vector_clock import ScopedClock

_wsplit_ctr = [0]


def split_multi_waits(nc):
    """The walrus build in this container accepts at most ONE sync wait per
    instruction. Split any instruction carrying N>1 waits into (N-1)
    single-wait nops on the same engine immediately before it; the original
    instruction keeps one wait and all its updates."""
    for f in nc.m.functions:
        for bb in f.blocks:
            out = []
            changed = False
            for inst in bb.instructions:
                si = inst.sync_info
                waits = list(si.on_wait) if si is not None and si.on_wait else []
                if len(waits) > 1:
                    updates = list(si.on_update) if si.on_update else []
                    for w in waits[1:]:
                        _wsplit_ctr[0] += 1
                        nop = mybir.InstNoOp(
                            name=f"I-wsplit-{_wsplit_ctr[0]}", ins=[], outs=[]
                        )
                        nop.engine = inst.engine
                        nop.sync_info = mybir.SyncInfo(on_wait=[w], on_update=[])
                        out.append(nop)
                    inst.sync_info = mybir.SyncInfo(on_wait=[waits[0]], on_update=updates)
                    changed = True
                out.append(inst)
            if changed:
                bb.instructions = out
    return nc

B, S, D, H, DH = 4, 2048, 1024, 16, 64
HG = 2  # head groups (tensor-parallel)
LD = D // HG  # 512 local model-dim slice
LH = H // HG  # 8 local heads
N_CORES = B * HG
SCALE = float(DH) ** -0.5
NEG_INF = -1e30

FP = mybir.dt.float32
FPR = mybir.dt.float32r
BF = mybir.dt.bfloat16

KC = D // 128  # 8 contraction chunks (projections)
MC = LD // 128  # 4 row chunks of the local dim (= head pairs)
SC = S // 128  # 16 seq chunks of 128
QT = S // 512  # 4 query tiles of 512
Exp = mybir.ActivationFunctionType.Exp
E1 = DH + 1  # per-head V stride incl. ones column


def _fr(ap):
    return ap.bitcast(FPR)


class SplitDrainTileContext(tile.TileContext):
    """The walrus build in this container rejects a Drain instruction with
    more than one sync wait; gate the tail drain with single-wait nops."""

    def _drain_and_barrier(self, tick_clock, wait_clock):
        nc = self.nc
        probe = nc.sync.nop()
        wait_clock.add_sem_waits(
            probe.ins, ScopedClock({None: tick_clock.global_clock})
        )
        si = probe.ins.sync_info
        waits = list(si.on_wait) if si is not None and si.on_wait else []
        updates = list(si.on_update) if si is not None and si.on_update else []
        if len(waits) > 1:
            probe.ins.sync_info = mybir.SyncInfo(on_wait=[waits[0]], on_update=updates)
            for w in waits[1:]:
                n2 = nc.sync.nop()
                n2.ins.sync_info = mybir.SyncInfo(on_wait=[w], on_update=[])
        nc.sync.drain()
        nc.all_engine_barrier()
        popped = nc._tile_sem_poison_stack.pop()
        assert popped is self._sem_poison
        nc.clear_and_free_semaphores(list(self.sems.allocated().values()))
        nc.all_engine_barrier()


def build_nc(for_hw=True):
    nc = bass.Bass(trn_type="TRN2")
    xT = nc.dram_tensor("xT", [D, S], FP, kind="ExternalInput").ap()
    wq = nc.dram_tensor("wq", [D, LD], FP, kind="ExternalInput").ap()
    wk = nc.dram_tensor("wk", [D, LD], FP, kind="ExternalInput").ap()
    wv = nc.dram_tensor("wv", [D, LD], FP, kind="ExternalInput").ap()
    wo = nc.dram_tensor("wo", [LD, D], FP, kind="ExternalInput").ap()
    kbias = nc.dram_tensor("kbias", [128, SC], FP, kind="ExternalInput").ap()
    ones_b = nc.dram_tensor("ones_b", [128, LH], BF, kind="ExternalInput").ap()
    out = nc.dram_tensor("out", [S, D], FP, kind="ExternalOutput").ap()

    with SplitDrainTileContext(nc) as tc:
        _body(tc, xT, wq, wk, wv, wo, kbias, ones_b, out)
    if for_hw:
        split_multi_waits(nc)
    return nc


def _body(tc, xT, wq, wk, wv, wo, kbias, ones_b, out):
    nc = tc.nc
    with (
        tc.tile_pool(name="pers", bufs=1) as pers,
        tc.tile_pool(name="pt", bufs=3) as pt_pool,
        tc.tile_pool(name="rs", bufs=2) as rs_pool,
        tc.tile_pool(name="ot", bufs=4) as ot_pool,
        tc.tile_pool(name="psmm", bufs=1, space="PSUM") as psmm,
    ):
        qt = [pers.tile([128, S], FP, tag=f"qt{m}", name=f"qt{m}") for m in range(MC)]
        kt = [pers.tile([128, S], FP, tag=f"kt{m}", name=f"kt{m}") for m in range(MC)]
        vt = [pers.tile([128, LH * E1], BF, tag=f"v{s}", name=f"v{s}") for s in range(SC)]
        att = [pers.tile([128, S], FP, tag=f"at{m}", name=f"at{m}") for m in range(MC)]
        biasT = pers.tile([128, SC], FP, tag="biasT")

        with tc.tile_critical():
            nc.gpsimd.load_library(library_config.attn)
        nc.sync.dma_start(biasT[:], kbias[:])
        for s in range(SC):
            # fill each head's ones column of V_aug straight from DRAM
            dst = vt[s][:].rearrange("p (h e) -> p h e", e=E1)[:, :, DH : DH + 1]
            nc.sync.dma_start(dst, ones_b[:, 0:LH].unsqueeze(2))

        # ---- stage A: projections ----
        with (
            tc.tile_pool(name="xt", bufs=1) as xt_pool,
            tc.tile_pool(name="w", bufs=1) as w_pool,
        ):
            def load_w(wdram, cast=True):
                wts = [
                    w_pool.tile([128, LD], FP, tag=f"w{k}", name=f"w{k}")
                    for k in range(KC)
                ]
                for k in range(KC):
                    src = wdram[k * 128 : (k + 1) * 128, :]
                    nc.sync.dma_start(_fr(wts[k][:]), _fr(src))
                return wts

            def v_pass(xts, half, wts):
                for sc in range(SC // 2):
                    s_idx = half * (SC // 2) + sc
                    ps = psmm.tile([128, 512], FP, tag="ps", name="ps", bufs=2)
                    j, off = sc // 4, (sc % 4) * 128
                    for k in range(KC):
                        nc.tensor.matmul(
                            ps[:],
                            _fr(xts[k][j][:, off : off + 128]),
                            _fr(wts[k][:]),
                            start=(k == 0),
                            stop=(k == KC - 1),
                        )
                    src = ps[:].rearrange("p (h e) -> p h e", h=LH)
                    dst = vt[s_idx][:].rearrange("p (h e) -> p h e", e=E1)[:, :, 0:DH]
                    nc.vector.tensor_copy(dst, src)

            def qk_pass(xts, half, wts, dstT, ms):
                for m in ms:
                    for q2 in range(2):
                        ps = psmm.tile([128, 512], FP, tag="ps", name="ps", bufs=2)
                        for k in range(KC):
                            nc.tensor.matmul(
                                ps[:],
                                _fr(wts[k][:, m * 128 : (m + 1) * 128]),
                                _fr(xts[k][q2][:]),
                                start=(k == 0),
                                stop=(k == KC - 1),
                            )
                        qlo = half * 1024 + q2 * 512
                        nc.vector.tensor_copy(_fr(dstT[m][:, qlo : qlo + 512]), ps[:])

            for half in range(2):
                # two 512-col slices per contraction chunk: the first V matmul
                # needs only the j=0 slices (2MB) instead of the full half (4MB)
                xts = [
                    [
                        xt_pool.tile([128, 512], FP, tag=f"xt{k}_{j}", name=f"xt{k}_{j}")
                        for j in range(2)
                    ]
                    for k in range(KC)
                ]
                for j in range(2):
                    for k in range(KC):
                        lo = half * 1024 + j * 512
                        nc.sync.dma_start(
                            _fr(xts[k][j][:]),
                            _fr(xT[k * 128 : (k + 1) * 128, lo : lo + 512]),
                        )
                if half == 0:
                    # V first (stage B's AV loop hits half-1 V chunks first)
                    v_pass(xts, half, load_w(wv))
                    qk_pass(xts, half, load_w(wq), qt, range(MC))
                    qk_pass(xts, half, load_w(wk), kt, range(MC))
                else:
                    # finish pair m=0 first so stage B starts while A finishes
                    wq_t = load_w(wq)
                    qk_pass(xts, half, wq_t, qt, [0])
                    wk_t = load_w(wk)
                    qk_pass(xts, half, wk_t, kt, [0])
                    v_pass(xts, half, load_w(wv))
                    qk_pass(xts, half, load_w(wq), qt, [1, 2, 3])
                    qk_pass(xts, half, load_w(wk), kt, [1, 2, 3])

        # ---- stages B+C ----
        with tc.tile_pool(name="wo", bufs=1) as wo_pool:
            wos = [
                wo_pool.tile([128, D], FP, tag=f"wo{j}", name=f"wo{j}")
                for j in range(MC)
            ]
            for j in range(MC):
                nc.sync.dma_start(_fr(wos[j][:]), _fr(wo[j * 128 : (j + 1) * 128, :]))

            def stage_c_slab(q):
                # output projection for one 512-query slab (4 chunks of 128)
                for qc in range(4 * q, 4 * (q + 1)):
                    for n in range(2):
                        ps = psmm.tile([128, 512], FP, tag="ps", name="psc", bufs=2)
                        for j in range(MC):
                            nc.tensor.matmul(
                                ps[:],
                                _fr(att[j][:, qc * 128 : (qc + 1) * 128]),
                                _fr(wos[j][:, n * 512 : (n + 1) * 512]),
                                start=(j == 0),
                                stop=(j == MC - 1),
                            )
                        ot = ot_pool.tile([128, 512], FP, tag="ot", name="ot")
                        nc.vector.tensor_copy(ot[:], ps[:])
                        nc.sync.dma_start(
                            out[qc * 128 : (qc + 1) * 128, n * 512 : (n + 1) * 512],
                            ot[:],
                        )

            # stage B: attention, one head-pair at a time
            for m in range(MC):
                hA, hB = 2 * m, 2 * m + 1
                for q in range(QT):
                    qs = slice(q * 512, (q + 1) * 512)
                    aA = psmm.tile([128, 512], FP, tag="aA", name="aA")
                    aB = psmm.tile([128, 512], FP, tag="aB", name="aB")
                    for kc in range(SC):
                        ks = slice(kc * 128, (kc + 1) * 128)
                        lg = psmm.tile([128, 1024], FP, tag="lg", name="lg", bufs=2)
                        nc.tensor.matmul(
                            lg[:, 0:512],
                            _fr(kt[m][0:64, ks]),
                            _fr(qt[m][0:64, qs]),
                            start=True,
                            stop=True,
                        )
                        nc.tensor.matmul(
                            lg[:, 512:1024],
                            _fr(kt[m][64:128, ks]),
                            _fr(qt[m][64:128, qs]),
                            start=True,
                            stop=True,
                        )
                        pt = pt_pool.tile([128, 1024], BF, tag="pt", name="pt")
                        nc.scalar.activation(
                            pt[:], lg[:], Exp, bias=biasT[:, kc : kc + 1], scale=SCALE
                        )
                        nc.tensor.matmul(
                            aA[0:65, :],
                            vt[kc][:, hA * E1 : (hA + 1) * E1],
                            pt[:, 0:512],
                            start=(kc == 0),
                            stop=(kc == SC - 1),
                            skip_group_check=True,
                        )
                        nc.tensor.matmul(
                            aB[0:65, :],
                            vt[kc][:, hB * E1 : (hB + 1) * E1],
                            pt[:, 512:1024],
                            start=(kc == 0),
                            stop=(kc == SC - 1),
                            skip_group_check=True,
                        )
                    for po, a_ps in ((0, aA), (64, aB)):
                        rs = rs_pool.tile([1, 512], FP, tag="rs", name="rs")
                        nc.vector.reciprocal(rs[:], a_ps[64:65, :])
                        bcs = rs_pool.tile([64, 512], FP, tag="bcs", name="bcs", bufs=2)
                        nc.gpsimd.partition_broadcast(bcs[:], rs[:])
                        nc.vector.tensor_tensor(
                            out=_fr(att[m][po : po + 64, qs]),
                            in0=a_ps[0:64, :],
                            in1=bcs[:],
                            op=mybir.AluOpType.mult,
                        )
                    if m == MC - 1:
                        stage_c_slab(q)



_nc = None


def get_nc():
    global _nc
    if _nc is None:
        _nc = build_nc()
    return _nc


def make_in_maps(x, mask, Wq, Wk, Wv, Wo):
    x = np.asarray(x, dtype=np.float32)
    mask = np.asarray(mask)
    Wq, Wk, Wv, Wo = (np.asarray(w, dtype=np.float32) for w in (Wq, Wk, Wv, Wo))
    in_maps = []
    for c in range(N_CORES):
        b, hg = c // HG, c % HG
        lo, hi = hg * LD, (hg + 1) * LD
        kb = np.where(mask[b], 0.0, NEG_INF).astype(np.float32)
        in_maps.append(
            {
                "xT": np.ascontiguousarray(x[b].T),
                "wq": np.ascontiguousarray(Wq[:, lo:hi]),
                "wk": np.ascontiguousarray(Wk[:, lo:hi]),
                "wv": np.ascontiguousarray(Wv[:, lo:hi]),
                "wo": np.ascontiguousarray(Wo[lo:hi, :]),
                "kbias": np.ascontiguousarray(kb.reshape(SC, 128).T),
                "ones_b": np.ones((128, LH), np.float32).astype(__import__("ml_dtypes").bfloat16),
            }
        )
    return in_maps


def kernel(x, mask, Wq, Wk, Wv, Wo):
    nc = get_nc()
    in_maps = make_in_maps(x, mask, Wq, Wk, Wv, Wo)
    res = run_bass_kernel_spmd(nc, in_maps, list(range(N_CORES)))
    outs = np.empty((B, S, D), dtype=np.float32)
    for b in range(B):
        outs[b] = res.results[2 * b]["out"] + res.results[2 * b + 1]["out"]
    return outs
